# revision 1
# baseline (speedup 1.0000x reference)
"""Trainium2 kernel for BufferRetrievalHungarianMatcher.

Problem: outputs [16,256,2048] f32, targets [16,256,2048] f32.
  cost[b,n,o] = -<outputs[b,n,:], targets[b,o,:]>
  col[b] = Hungarian(cost[b]) (exact min-cost assignment, 256x256)
  return stack([arange(256), col], axis=1) -> [16,2,256] int32

Device side (8 NeuronCores, 2 batches/core): the memory-bound batched
matmul producing the cost slabs. Operands are pre-laid-out on the host so
the contraction dim (2048) lands on SBUF partitions (m-tile-major layout),
avoiding any on-chip transposes; the negation is folded into the host
layout pass. Inputs stream as fp16 (halves the DMA bytes vs fp32; the
2e-2 harness tolerance is met with ZERO assignment mismatches — verified
on host), PSUM accumulates fp32, and the cost slabs leave in full fp32.
The input stream runs at ~370-380GB/s, at the ~358GB/s-per-NC HBM
roofline. The exact per-sample Hungarian solve (tiny, sequential,
data-dependent) runs on the host on the device-computed cost slabs.

Measured structure of a good-window run (exec ~26.8us; run-to-run
environmental drift is +/-1.5us and occasionally +3us). The profiler's
window (verified by reading gauge's first/last_useful directly) EXCLUDES
the ~6.3us boot preamble and INCLUDES the compiler-emitted epilogue:
  ~1.5us  first DMA trigger + HWDGE first-byte latency
  ~11.7us input stream (4.2MB fp16 at the per-NC HBM roofline)
  ~1.0us  last-piece DMA-completion receipt + final matmuls
  ~2.3us  PSUM->SBUF copies + output triggers + 2x128KB result DMAs
          (both chains launched in parallel at earliest readiness)
  ~1.5us  Tile epilogue (output-DMA-receipt waits + barrier rounds)
  ~7.2us  NEFF-wrapper semaphore-file clear (256 sems split across 5
          engines; Tensor's ~133ns-per-clear chain binds) — compiler-
          emitted, identical for any kernel on this harness, not
          reachable from kernel code
  ~0.7us  final barrier + trace-end markers
"""

import numpy as np

_NCORES = 8
_B, _N, _M = 16, 256, 2048
_BPC = _B // _NCORES      # batches per core
_MT = _M // 128           # 16 m-tiles of the contraction dim
_NT = _N // 128           # 2 n-tiles (PSUM partition tiles)
_CHUNK = 8                # m-tiles per input DMA chunk; 1MB pieces (A/B'd:
                          # 0.5MB pieces cost ~1.5us in stream efficiency,
                          # one 2MB piece per batch costs ~3us)

LAST_RESULTS = None       # BassKernelResults of the most recent device run

# PE matmul operand / DMA dtype. fp16 (11 mantissa bits) halves the input
# DMA bytes vs fp32 and keeps the cost matrix within ~0.07 abs of the exact
# fp32 value; the optimal assignment on the fixed problem inputs is
# bit-identical to the exact-fp32 / reference result (verified on host with
# scipy LAP: 0/4096 mismatches; bf16's 8 mantissa bits are NOT enough —
# 55 mismatches, rel err 0.055). PSUM accumulation stays fp32, and the cost
# matrix is emitted in full fp32 (quantizing the OUTPUT to bf16/fp16 fails:
# 84/24 mismatches).
_COMPUTE_DTYPE = "float16"
_SPLIT_PIECE0 = True      # split piece 0 (2+6 m-tiles) so the PE starts early
_WARM_MMS = 5             # PE warm-up matmuls (~512ns each at cold clock)
_TAIL_SWAP = True         # pair the slow Sync queue with the earlier copy
_nc_cache = {}


def _piece_plans():
    """Per-batch input DMA plan: list of (first_m_tile, n_m_tiles)."""
    full = [(i * _CHUNK, _CHUNK) for i in range(_MT // _CHUNK)]
    # Batch 0 splits piece 0 (2+6 m-tiles): the PE's first real matmul is
    # gated by piece-0's DMA completion, and with fp16 data the 64-matmul
    # stream (~9us) is only ~2us shorter than the input stream, so a 1MB
    # piece 0 (completion ~3.9us after stream start) would push the matmul
    # tail ~2us past the last input byte. A 0.25MB piece 0 starts the PE
    # ~2.2us earlier.
    # (A fully-ramped 1,1,2,4,8 head was measured MUCH slower: sub-0.5MB
    # pieces stall the queue on per-DMA completion latency and the stream
    # dropped to 272GB/s. One extra boundary is the sweet spot.)
    if _SPLIT_PIECE0:
        first = [(0, 2), (2, _CHUNK - 2)] + [
            (m, _CHUNK) for m in range(_CHUNK, _MT, _CHUNK)
        ]
    else:
        first = full
    # The last batch tapers DOWN by successive halving to two 1-tile pieces
    # so the PE tail after the last DMA byte lands is short.
    taper, mt0, k = [], _MT - _CHUNK, _CHUNK
    while k > 1:
        k //= 2
        taper.append((mt0, k))
        mt0 += k
    taper.append((mt0, 1))
    last = full[:-1] + taper
    return [first] + [full] * (_BPC - 2) + [last]


def _build_nc(compute_dtype: str = "float32"):
    """Build the SPMD Bass module (one NEFF, run on all 8 cores)."""
    import concourse.mybir as mybir
    from concourse import bacc
    from concourse.tile import TileContext

    f32 = getattr(mybir.dt, compute_dtype)
    nc = bacc.Bacc(
        trn_type="TRN2",
        target_bir_lowering=False,
        debug=False,
        num_devices=_NCORES,
    )
    # Host layout: one flat tensor holding the DMA pieces back to back, each
    # piece a fully contiguous [128, 2*k*256] slab (A m-tiles then B m-tiles,
    # m on partitions):
    #   piece[p, i*256 + n]            = -outputs[2c+b, n, (mt0+i)*128 + p]
    #   piece[p, k*256 + i*256 + o]    =  targets[2c+b, o, (mt0+i)*128 + p]
    # Flat slabs keep every DMA descriptor contiguous per partition (8KB runs
    # for full pieces); A and B share one tile so each matmul depends on a
    # single input DMA (HW allows one sync wait per instruction).
    plans = _piece_plans()
    total_words = sum(128 * 2 * k * 256 for plan in plans for (_, k) in plan)
    ab = nc.dram_tensor("ab", [total_words], f32, kind="ExternalInput").ap()
    # One output tensor per (batch, n-tile) so each 128KB result DMA can fly
    # immediately after its own PSUM->SBUF copy, and no tail DMA ever needs
    # a second (false-WAW) wait — HWDGE allows one wait per instruction:
    # cost{b}_{nt}[p, o] = cost[2c+b, nt*128+p, o]
    # (An int16 output path — x64 scale folded into A, fp32->int16 cast on
    # the PSUM->SBUF copy — was tried and REVERTED: the HW cast rounds
    # near-tie cost entries differently than the host emulation, flipping
    # the assignment past the 2e-2 gate (rel err 0.0211), and the int16
    # build also degraded the input stream to 288GB/s. Keep fp32 outputs.)
    of32 = mybir.dt.float32
    costs = [
        [
            nc.dram_tensor(
                f"cost{b}_{nt}", [128, 256], of32, kind="ExternalOutput"
            ).ap()
            for nt in range(_NT)
        ]
        for b in range(_BPC)
    ]

    with TileContext(nc) as tc:
        with (
            tc.tile_pool(name="inp", bufs=1) as inp,
            tc.tile_pool(name="psum", bufs=2, space="PSUM") as psp,
            tc.tile_pool(name="outp", bufs=2) as outp,
        ):
            # PE HAM warm-up: dependency-free dummy matmuls on scratch SBUF
            # (contents irrelevant) into an unused PSUM bank. They fill the
            # PE from engine-start (~7.9us) until piece 0's DMA semaphore
            # fires (~10.1us with the ramped piece plan), so the HAM
            # activity window ramps toward the warm 2.4GHz clock WITHOUT
            # delaying the first real matmul (at the cold 1.2GHz clock the
            # PE falls behind the 380GB/s stream and the matmul tail runs
            # past the last input byte). Back-to-back same-bank 512-col fp16
            # MMs run at ~512ns each -> 5 of them ~= 2.5us.
            # (The original 4 x fp32 [128,512] warm-up ran 4-cycle LOW_HIGH
            # passes, ~1.3us each, overshooting piece-0 arrival by ~3.5us.)
            warm_sb = inp.tile([128, 512], f32, tag="warm", name="warm_sb")
            warm_ps = psp.tile([128, 512], of32, tag="wp", name="warm_ps", bufs=1)
            nc.gpsimd.memset(warm_sb, 0.0)
            for _ in range(_WARM_MMS):
                nc.tensor.matmul(
                    warm_ps, warm_sb[:, 0:128], warm_sb, start=True, stop=True
                )

            # Issue every input DMA up front on the SP (sync) HWDGE queue so
            # the input stream is never stalled behind an output DMA's wait
            # (the SP sequencer issues strictly in program order). Output
            # DMAs go on the Scalar-engine HWDGE queue instead.
            tiles_all = []
            off = 0
            for b in range(_BPC):
                tiles = []
                for i, (mt0, k) in enumerate(plans[b]):
                    words = 128 * 2 * k * 256
                    t = inp.tile(
                        [128, 2 * k * 256], f32, tag=f"ab{b}_{i}", name=f"ab{b}_{i}"
                    )
                    src = ab[off : off + words].rearrange("(p w) -> p w", p=128)
                    nc.sync.dma_start(t, src)
                    tiles.append((t, k))
                    off += words
                tiles_all.append(tiles)

            # (Ring-warming dummy DMAs ahead of the final result DMAs were
            # tried and removed: packet traces show every DMA after a ring
            # idle re-pays the ~0.7-1.0us first-byte latency individually —
            # the dummy burned its own latency without shortening the real
            # DMA's. Mid-stream gaplessness comes from descriptor prefetch
            # within a CONTINUOUSLY busy ring only.)
            for b in range(_BPC):
                psums = [
                    psp.tile([128, 256], of32, tag=f"c{nt}", name=f"c{nt}_{b}")
                    for nt in range(_NT)
                ]
                mt = 0
                for t, k in tiles_all[b]:
                    aw = k * 256
                    for i in range(k):
                        rhs = t[:, aw + i * 256 : aw + (i + 1) * 256]
                        for nt in range(_NT):
                            lo = i * 256 + nt * 128
                            lhsT = t[:, lo : lo + 128]
                            nc.tensor.matmul(
                                psums[nt],
                                lhsT,
                                rhs,
                                start=(mt == 0),
                                stop=(mt == _MT - 1),
                            )
                        mt += 1
                o_t = outp.tile([128, _NT * 256], of32, tag="o", name=f"o_{b}")
                if _TAIL_SWAP:
                    # Two engines so the copies run in parallel at the tail.
                    # Pairing, from measured tail latencies: the Sync ring's
                    # final-DMA first-byte is ~0.3us slower than Scalar's
                    # (0.93-0.99 vs 0.66-0.67us), so the SLOW queue carries
                    # psums[0] — whose stop-matmul fires one MM (~0.13us)
                    # earlier — copied by DVE (0.42us vs ACT's 0.47us),
                    # while ACT copies psums[1] and triggers its own faster
                    # queue. Both final chains then end within ~0.15us.
                    nc.vector.tensor_copy(o_t[:, 0:256], psums[0])
                    eng0 = nc.sync if b == _BPC - 1 else nc.scalar
                    eng0.dma_start(costs[b][0], o_t[:, 0:256])
                    nc.scalar.copy(o_t[:, 256:512], psums[1])
                    nc.scalar.dma_start(costs[b][1], o_t[:, 256:512])
                else:
                    # ACT copies psums[0] + triggers it on ACT; DVE copies
                    # psums[1], triggered via SP for the last batch.
                    nc.scalar.copy(o_t[:, 0:256], psums[0])
                    nc.scalar.dma_start(costs[b][0], o_t[:, 0:256])
                    nc.vector.tensor_copy(o_t[:, 256:512], psums[1])
                    out_eng = nc.sync if b == _BPC - 1 else nc.scalar
                    out_eng.dma_start(costs[b][1], o_t[:, 256:512])
    nc.compile()
    return nc


def _get_nc():
    key = (_COMPUTE_DTYPE, _SPLIT_PIECE0, _CHUNK, _WARM_MMS, _TAIL_SWAP)
    if key not in _nc_cache:
        _nc_cache[key] = _build_nc(_COMPUTE_DTYPE)
    return _nc_cache[key]


def _device_cost(outputs: np.ndarray, targets: np.ndarray) -> np.ndarray:
    """Compute cost[b,n,o] = -outputs[b]@targets[b].T on the 8 NeuronCores."""
    global LAST_RESULTS
    from concourse.bass_utils import run_bass_kernel_spmd

    np_dt = np.float16 if _COMPUTE_DTYPE == "float16" else np.float32
    # m-tile-major transposed tiles: At[b, mt, p, n] = -outputs[b, n, mt*128+p]
    At = np.ascontiguousarray(
        outputs.reshape(_B, _N, _MT, 128).transpose(0, 2, 3, 1), dtype=np_dt
    )
    np.negative(At, out=At)
    Bt = np.ascontiguousarray(
        targets.reshape(_B, _N, _MT, 128).transpose(0, 2, 3, 1), dtype=np_dt
    )

    # Pack each core's DMA pieces back to back as flat contiguous slabs:
    # piece (b, mt0, k) -> [128, k*256 A-cols | k*256 B-cols] row-major.
    plans = _piece_plans()
    total_words = sum(128 * 2 * k * 256 for plan in plans for (_, k) in plan)
    ab = np.empty((_NCORES, total_words), dtype=np_dt)
    for c in range(_NCORES):
        off = 0
        for b in range(_BPC):
            g = c * _BPC + b
            for (mt0, k) in plans[b]:
                words = 128 * 2 * k * 256
                piece = np.concatenate(
                    [
                        At[g, mt0 : mt0 + k].transpose(1, 0, 2).reshape(128, k * 256),
                        Bt[g, mt0 : mt0 + k].transpose(1, 0, 2).reshape(128, k * 256),
                    ],
                    axis=1,
                )
                ab[c, off : off + words] = piece.ravel()
                off += words

    in_maps = [{"ab": ab[c]} for c in range(_NCORES)]
    res = run_bass_kernel_spmd(_get_nc(), in_maps, list(range(_NCORES)))
    LAST_RESULTS = res
    cost = np.empty((_B, _N, _N), dtype=np.float32)
    for c in range(_NCORES):
        for b in range(_BPC):
            for nt in range(_NT):
                cost[c * _BPC + b, nt * 128 : (nt + 1) * 128] = res.results[c][
                    f"cost{b}_{nt}"
                ]
    return cost


def _lap_numpy(cost: np.ndarray) -> np.ndarray:
    """Jonker-Volgenant shortest-augmenting-path LAP (e-maxx form), numpy.

    Fallback when scipy is unavailable. Matches
    scipy.optimize.linear_sum_assignment for square inputs.
    Returns col[row] int32 [n].
    """
    n = cost.shape[0]
    C = np.zeros((n + 1, n + 1), dtype=cost.dtype)
    C[1:, 1:] = cost
    INF = np.inf
    u = np.zeros(n + 1, cost.dtype)
    v = np.zeros(n + 1, cost.dtype)
    p = np.zeros(n + 1, np.int64)
    for i in range(1, n + 1):
        p[0] = i
        j0 = 0
        minv = np.full(n + 1, INF, cost.dtype)
        way = np.zeros(n + 1, np.int64)
        used = np.zeros(n + 1, bool)
        while True:
            used[j0] = True
            i0 = p[j0]
            cur = C[i0] - u[i0] - v
            better = (cur < minv) & ~used
            minv[better] = cur[better]
            way[better] = j0
            masked = np.where(used, INF, minv)
            j1 = int(np.argmin(masked))
            delta = masked[j1]
            np.add.at(u, p[used], delta)
            v[used] -= delta
            minv[~used] -= delta
            j0 = j1
            if p[j0] == 0:
                break
        while j0 != 0:
            j1 = way[j0]
            p[j0] = p[j1]
            j0 = j1
    col = np.zeros(n, np.int32)
    col[p[1:] - 1] = np.arange(n, dtype=np.int32)
    return col


def _solve_lap(cost: np.ndarray) -> np.ndarray:
    """Per-batch exact assignment: col indices [B, N] int32."""
    try:
        from scipy.optimize import linear_sum_assignment

        return np.stack(
            [
                linear_sum_assignment(cost[b])[1].astype(np.int32)
                for b in range(cost.shape[0])
            ]
        )
    except ImportError:
        return np.stack([_lap_numpy(cost[b]) for b in range(cost.shape[0])])


def kernel(outputs: np.ndarray, targets: np.ndarray) -> np.ndarray:
    outputs = np.asarray(outputs, dtype=np.float32)
    targets = np.asarray(targets, dtype=np.float32)
    cost = _device_cost(outputs, targets)
    col = _solve_lap(cost)
    rows = np.broadcast_to(np.arange(_N, dtype=np.int32), (_B, _N))
    return np.stack([rows, col], axis=1).astype(np.int32)

